# revision 35
# baseline (speedup 1.0000x reference)
"""AntiAliasInterpolation2d Trainium kernel.

out[n,i,j,c] = sum_{dy,dx} g[dy]*g[dx] * x[n, 4i+dy-6, 4j+dx-6, c]   (zero pad)

i.e. a separable 13-tap Gaussian blur evaluated only on the stride-4 output
grid (the nearest-neighbor downsample of the reference picks blurred[4i,4j]).

Per core (batch shard of 4 images):
  vertical:   t1[i, w] = sum_h AB[h, i] * x[h, w]      (TensorE matmul,
              AB is the banded 512x128 matrix AB[h,i] = g[h-4i+6])
  narrow:     ACT copies t1 PSUM -> SBUF as bf16 into a tile with 8
              zero-guard columns each side (tolerance is 2e-2; bf16
              keeps rel err ~4e-3)
  horizontal: exploits kernel symmetry g[6-d]==g[6+d]:
                u_d[j]  = t1[4j-d] + t1[4j+d]   (6 DVE tensor_tensor adds,
                          bf16 2x mode; guards make every op full-range)
                out[j]  = g[6]*t1[4j] + sum_d g[6+d]*u_d[j]
                          (DVE: 4x-mode tensor_scalar + 6 stt MACs)
              accumulated in bf16 and DMA'd out bf16 (halves the output
              HBM write; the host widens back to f32)

Steady-state throughput is HBM-DMA-bound (~13.0 MB of HBM traffic per
core iteration at ~360 GB/s/core; measured 35.2-35.7us/iter vs a
35.3us bound at the 368 GB/s cost-model derate). For timing NEFFs
(repeats>1) the body holds up to 50 whole kernel executions per For_i
iteration: For_i places an all-engine barrier at each loop iteration,
and consecutive executions can only overlap (the tap tail hiding under
the next execution's input DMAs) within one loop body. The guard-zero
memsets run once in a prologue — the casts only write the tile
interior, so guards stay valid across iterations.

Things measured slower on HW than the baseline they modify:
  - pair-adds on Pool/GpSimd (n_pool>0): cost model says ~760ns/op but
    HW is far worse (42.9us vs 36.6us per iteration)
  - staggered_reset For_i instead of body unrolling (52.5us)
  - 13 plain stt taps straight from PSUM f32 (the previous scheme):
    DVE stream ~35.5us/iter paced right at the DMA wall (46.9us/iter
    with per-iteration barriers, 37.2us with ubody=16)

Built on bacc.Bacc: its generate_event_semaphores pass splits Tile's
multi-semaphore waits into EventSemaphore instructions (this walrus build
allows at most one semaphore wait per regular instruction).
"""

import numpy as np

try:
    import concourse.bass as bass
except ImportError:  # pragma: no cover
    import sys

    sys.path.insert(0, "/opt/trn_rl_repo")
    import concourse.bass as bass

import concourse.mybir as mybir
from concourse import bacc, tile
from concourse.bass_utils import run_bass_kernel_spmd

N_CORES = 8
N_PER_CORE = 4          # 32 images / 8 cores
H = W = 512
C = 3
OH = OW = 128
KSIZE = 13
KA = 6
SIGMA = 1.5
PAD = 8                 # zero-guard columns each side of the bf16 t1 copy
WPAD = W + 2 * PAD


def _gauss_norm() -> np.ndarray:
    r = np.arange(KSIZE, dtype=np.float32)
    g = np.exp(-((r - np.float32(KA)) ** 2) / np.float32(2.0 * SIGMA * SIGMA))
    return (g / g.sum()).astype(np.float32)


def _band_matrix() -> np.ndarray:
    """AB[h, i] = g[h - 4i + 6], zero outside the band."""
    g = _gauss_norm()
    ab = np.zeros((H, OH), dtype=np.float32)
    for i in range(OH):
        for dy in range(KSIZE):
            h = 4 * i + dy - KA
            if 0 <= h < H:
                ab[h, i] = g[dy]
    return ab


def _tap_ranges():
    """For each dx: (j0, j1, r, q) s.t. src w-index = 4*(j+q) + r for j in [j0,j1)."""
    taps = []
    for dx in range(KSIZE):
        off = dx - KA
        j0 = 0 if off >= 0 else (-off + 3) // 4  # ceil(-off/4)
        j1 = min(OW, (W - 1 - off) // 4 + 1)
        r = off % 4
        q = (off - r) // 4
        taps.append((dx, j0, j1, r, q))
    return taps


def build_nc(
    repeats: int = 1,
    n_chunks: int = 4,
    sbuf_taps: int = 0,
    dma_only: int = 0,
    tail_split: int = 0,
    unroll: int = 0,
    stag: int = 0,
    ubody: int = 0,
    obf: int = 1,
    split_taps: int = 1,
    n_pool: int = 0,
    rowpack: int = 0,
) -> bass.Bass:
    """repeats>1 re-runs the whole per-core program (for timing benchmarks).
    n_chunks: x DMAs per image (1, 2, or 4 h-blocks per DMA).
    sbuf_taps: copy t1 PSUM->SBUF on ACT first (measured slower on HW).
    dma_only: benchmark variant that skips all compute.
    tail_split: stream the LAST image in bank-aligned W-thirds so most of
      its tap work overlaps the final DMAs. Off by default: the DVE is
      still draining image 2's taps when image 3's early thirds land, so
      the split only adds op overhead (cost model: 52.5us vs 51.3us)."""
    nc = bacc.Bacc()
    f32 = mybir.dt.float32
    bf16 = mybir.dt.bfloat16
    # float32r: same 32-bit storage, but the PE streams it at 1 cycle/row
    # (plain float32 matmuls decompose into 2 half-rate passes = 4x slower)
    f32r = mybir.dt.float32r
    x = nc.declare_dram_parameter("x", [N_PER_CORE, H, W, C], f32r, isOutput=False)
    ab = nc.declare_dram_parameter("ab", [H, OH], f32r, isOutput=False)
    # obf: store the output as bf16 (halves the output HBM write traffic;
    # the host widens back to f32 — tolerance is 2e-2, bf16 err ~3e-3)
    out = nc.declare_dram_parameter(
        "out", [N_PER_CORE, OH, OW, C], bf16 if obf else f32, isOutput=True
    )

    g = _gauss_norm()
    taps = _tap_ranges()
    # full-coverage tap (dx=6) first so it can initialize the accumulator
    taps.sort(key=lambda t: t[0] != KA)

    with tile.TileContext(nc) as tc:
        with (
            tc.tile_pool(name="const", bufs=1) as cpool,
            tc.tile_pool(name="xp", bufs=1) as xpool,
            tc.tile_pool(name="op", bufs=1) as opool,
            tc.tile_pool(name="ps", bufs=2, space="PSUM") as pspool,
        ):
            # banded vertical matrix: sbuf [p=h%128, (k, i)] from dram
            # [(k p), i]; issued on the ACT HWDGE queue so it doesn't delay
            # the first x chunk at the head of the SP queue
            ab_s = cpool.tile([128, 4 * OH], f32r)
            nc.scalar.dma_start(
                out=ab_s[:].rearrange("p (k i) -> p k i", k=4),
                in_=ab.rearrange(
                    "(p k) i -> p k i" if rowpack else "(k p) i -> p k i",
                    p=128,
                ),
            )

            t1s_tiles = {}
            if split_taps and not dma_only:
                # one-time prologue: zero the guard columns of each image's
                # padded bf16 t1 tile. The per-iteration ACT casts only write
                # the interior, so the guards stay zero for the whole program.
                for n in range(N_PER_CORE):
                    t1s_tiles[n] = opool.tile(
                        [128, WPAD * C], bf16, tag=f"t1s{n}", name=f"t1s{n}"
                    )
                    nc.vector.memset(t1s_tiles[n][:, : PAD * C], 0.0)
                    nc.vector.memset(t1s_tiles[n][:, (PAD + W) * C :], 0.0)

            kb = 4 // n_chunks  # h-blocks per DMA

            def emit_image(n):
                # per-chunk DMAs: matmuls for a chunk start as soon as it
                # lands instead of waiting for the whole 3MB image;
                # dedicated tiles so the DMAs have no WAR deps
                xts = []
                for ck in range(n_chunks):
                    xtk = xpool.tile(
                        [128, kb * W * C], f32r, tag=f"xt{n}k{ck}", name=f"xt{n}k{ck}"
                    )
                    # rowpack: partition p holds kb CONSECUTIVE DRAM rows —
                    # one contiguous kb*6KB descriptor per partition instead
                    # of kb separate 6KB ones (ab is row-permuted to match)
                    src = (
                        "(ck p b) w c -> ck p b (w c)"
                        if rowpack
                        else "(ck b p) w c -> ck p b (w c)"
                    )
                    nc.sync.dma_start(
                        out=xtk[:].rearrange("p (b f) -> p b f", b=kb),
                        in_=x[n].rearrange(src, p=128, b=kb)[ck],
                    )
                    xts.append(xtk)

                if dma_only:
                    ot = opool.tile(
                        [128, OW * C], bf16 if obf else f32, tag=f"ot{n}", name=f"ot{n}"
                    )
                    nc.vector.tensor_copy(ot[:], xts[0][:, : OW * C].bitcast(f32))
                    nc.scalar.dma_start(
                        out=out[n].rearrange("i j c -> i (j c)"), in_=ot[:]
                    )
                    return

                # vertical blur via matmul, on the INTERLEAVED (w c) layout:
                # every column of x is blurred independently, so rhs can be
                # contiguous 512-element slices (PE streams at line rate;
                # strided rhs would throttle the XBUS). t1 free index is
                # m = w*3 + c.
                t1 = pspool.tile([128, C * W], f32, tag="t1", name=f"t1_{n}")
                for k in range(4):
                    lhsT = ab_s[:, k * OH : (k + 1) * OH]
                    xvk = xts[k // kb][:].rearrange("p (b f) -> p b f", b=kb)[
                        :, k % kb
                    ]
                    for s in range(C):
                        nc.tensor.matmul(
                            t1[:, s * W : (s + 1) * W],
                            lhsT,
                            xvk[:, s * W : (s + 1) * W],
                            start=(k == 0),
                            stop=(k == 3),
                        )

                if split_taps:
                    # Horizontal blur v2, exploiting the symmetric kernel
                    # (g[6-d] == g[6+d]) and a zero-padded bf16 copy of t1:
                    #   u_d[j] = t1[4j-d] + t1[4j+d]       (6 tensor_tensor
                    #       adds, split Pool/DVE; guard zeros make every op
                    #       full-range so there are no edge cases)
                    #   out[j] = g[6]*t1[4j] + sum_d g[6+d]*u_d[j]
                    #       (DVE: one 4x-mode tensor_scalar + 6 stt MACs,
                    #       accumulated in bf16 and DMA'd directly)
                    # ACT narrows t1 PSUM->SBUF bf16 (Pool has no PSUM port),
                    # one 512-wide PSUM-bank slice at a time.
                    t1s = t1s_tiles[n]
                    for s in range(C):
                        nc.scalar.copy(
                            t1s[:, PAD * C + s * W : PAD * C + (s + 1) * W],
                            t1[:, s * W : (s + 1) * W],
                        )

                    # [p, r, wq, c] with w_padded = 4*wq + r
                    t1r = t1s[:].rearrange("p (wq r c) -> p r wq c", r=4, c=C)

                    def src_at(off):
                        # view of t1[4j + off] for j in [0, OW)
                        wp = PAD + off
                        return t1r[:, wp % 4, wp // 4 : wp // 4 + OW]

                    acc = opool.tile(
                        [128, OW * C], bf16 if obf else f32,
                        tag=f"acc{n}", name=f"acc{n}",
                    )
                    av = acc[:].rearrange("p (j c) -> p j c", c=C)

                    us = {}
                    # n_pool = how many of the 6 pair-adds run on Pool
                    dve_pairs = tuple(range(1, 7 - n_pool))
                    for d in range(1, 7):
                        u = opool.tile([128, OW * C], bf16, tag=f"u{n}d{d}", name=f"u{n}d{d}")
                        eng = nc.vector if d in dve_pairs else nc.gpsimd
                        eng.tensor_tensor(
                            u[:].rearrange("p (j c) -> p j c", c=C),
                            src_at(-d),
                            src_at(d),
                            mybir.AluOpType.add,
                        )
                        us[d] = u

                    nc.vector.tensor_scalar(
                        av, src_at(0), float(g[KA]), None, mybir.AluOpType.mult
                    )
                    # DVE-computed pairs first so the chain rarely stalls on Pool
                    for d in list(dve_pairs) + [d for d in range(1, 7) if d not in dve_pairs]:
                        nc.vector.scalar_tensor_tensor(
                            acc[:],
                            us[d][:],
                            float(g[KA + d]),
                            acc[:],
                            mybir.AluOpType.mult,
                            mybir.AluOpType.add,
                        )
                    nc.scalar.dma_start(
                        out=out[n].rearrange("i j c -> i (j c)"), in_=acc[:]
                    )
                    return

                if sbuf_taps:
                    # PSUM -> SBUF via ACT so the DVE taps run all-SBUF
                    # (2x_2p mode in the cost model; measured slower on HW)
                    t1s = opool.tile(
                        [128, C * W], f32, tag=f"t1s{n}", name=f"t1s{n}"
                    )
                    nc.scalar.copy(t1s[:], t1[:])
                    tap_src = t1s
                else:
                    tap_src = t1

                # horizontal blur: 13 strided MACs on DVE
                # src index m = w*3 + c with w = 4u + r -> view [p, r, u, c]
                # (c innermost: each AP step covers a contiguous 12B triple)
                t1v = tap_src[:].rearrange("p (u r c) -> p r u c", r=4, c=C)
                ot = opool.tile([128, OW * C], f32, tag=f"ot{n}", name=f"ot{n}")
                ov = ot[:].rearrange("p (j c) -> p j c", c=C)

                first = True
                for dx, j0, j1, r, q in taps:
                    src = t1v[:, r, j0 + q : j1 + q]
                    dst = ov[:, j0:j1]
                    if first:
                        first = False
                        nc.vector.tensor_scalar(
                            dst, src, float(g[dx]), None, mybir.AluOpType.mult
                        )
                    else:
                        nc.vector.scalar_tensor_tensor(
                            dst,
                            src,
                            float(g[dx]),
                            dst,
                            mybir.AluOpType.mult,
                            mybir.AluOpType.add,
                        )

                # out DMA on the ACT HWDGE queue: its wait on the taps must
                # not block dispatch of later x DMAs on the SP queue
                if obf:
                    # narrow to bf16 on the (otherwise idle) ACT engine so the
                    # HBM write is half-width
                    otb = opool.tile([128, OW * C], bf16, tag=f"otb{n}", name=f"otb{n}")
                    nc.scalar.copy(otb[:], ot[:])
                    nc.scalar.dma_start(
                        out=out[n].rearrange("i j c -> i (j c)"), in_=otb[:]
                    )
                else:
                    nc.scalar.dma_start(
                        out=out[n].rearrange("i j c -> i (j c)"), in_=ot[:]
                    )

            def emit_image_tailsplit(n):
                # Last image of the stream: DMA it in 12 bank-aligned
                # W-thirds (third-major), and run the taps in two phases.
                # Phase A (j < 84) reads only PSUM banks 0-1 (m <= 1019)
                # so it overlaps the final third's DMAs + matmuls; only
                # phase B (j >= 84, ~1/3 of the tap work) trails the last
                # byte.
                JB = 84
                xts = {}
                for s in range(C):
                    for k in range(4):
                        t = xpool.tile(
                            [128, W], f32r, tag=f"xs{n}s{s}k{k}", name=f"xs{n}s{s}k{k}"
                        )
                        nc.sync.dma_start(
                            out=t[:],
                            in_=x[n].rearrange("(k p) w c -> k p (w c)", p=128)[k][
                                :, 512 * s : 512 * (s + 1)
                            ],
                        )
                        xts[(s, k)] = t

                t1 = pspool.tile([128, C * W], f32, tag="t1", name=f"t1_{n}")
                t1v = t1[:].rearrange("p (u r c) -> p r u c", r=4, c=C)
                ot = opool.tile([128, OW * C], f32, tag=f"ot{n}", name=f"ot{n}")
                ov = ot[:].rearrange("p (j c) -> p j c", c=C)

                def emit_taps(jlo, jhi):
                    first = True
                    for dx, j0, j1, r, q in taps:
                        jl, jh = max(j0, jlo), min(j1, jhi)
                        if jl >= jh:
                            continue
                        src = t1v[:, r, jl + q : jh + q]
                        dst = ov[:, jl:jh]
                        if first:
                            first = False
                            nc.vector.tensor_scalar(
                                dst, src, float(g[dx]), None, mybir.AluOpType.mult
                            )
                        else:
                            nc.vector.scalar_tensor_tensor(
                                dst,
                                src,
                                float(g[dx]),
                                dst,
                                mybir.AluOpType.mult,
                                mybir.AluOpType.add,
                            )

                for s in range(C):
                    for k in range(4):
                        nc.tensor.matmul(
                            t1[:, 512 * s : 512 * (s + 1)],
                            ab_s[:, k * OH : (k + 1) * OH],
                            xts[(s, k)][:],
                            start=(k == 0),
                            stop=(k == 3),
                        )
                    if s == 1:
                        emit_taps(0, JB)
                emit_taps(JB, OW)

                if obf:
                    otb = opool.tile([128, OW * C], bf16, tag=f"otb{n}", name=f"otb{n}")
                    nc.scalar.copy(otb[:], ot[:])
                    nc.scalar.dma_start(
                        out=out[n].rearrange("i j c -> i (j c)"), in_=otb[:]
                    )
                else:
                    nc.scalar.dma_start(
                        out=out[n].rearrange("i j c -> i (j c)"), in_=ot[:]
                    )

            def emit_all():
                for n in range(N_PER_CORE):
                    # tail_split = how many trailing images get the
                    # W-thirds streaming treatment
                    if dma_only or n < N_PER_CORE - tail_split:
                        emit_image(n)
                    else:
                        emit_image_tailsplit(n)

            if repeats == 1:
                emit_all()
            elif unroll:
                for _ in range(repeats):
                    emit_all()
            else:
                # unroll several whole kernel executions inside the For_i
                # body: For_i places an all-engine barrier at each loop
                # iteration, so consecutive executions only overlap (tail
                # under next execution's DMAs) within one body
                ub = ubody
                if ub == 0:
                    ub = next(
                        (u for u in (50, 32, 16, 8, 4, 2, 1) if repeats % u == 0), 1
                    )
                assert repeats % ub == 0, (repeats, ub)
                with tc.For_i(0, repeats // ub, 1, staggered_reset=bool(stag)):
                    for _ in range(ub):
                        emit_all()

    nc.finalize()
    return nc


_NC_CACHE = None


def _get_nc() -> bass.Bass:
    global _NC_CACHE
    if _NC_CACHE is None:
        _NC_CACHE = build_nc()
    return _NC_CACHE


def run(x: np.ndarray, trace: bool = False):
    """Returns (out [32,128,128,3] f32, exec_time_ns or None)."""
    x = np.ascontiguousarray(np.asarray(x, dtype=np.float32))
    assert x.shape == (N_CORES * N_PER_CORE, H, W, C), x.shape
    ab = _band_matrix()
    nc = _get_nc()
    in_maps = [
        {"x": x[i * N_PER_CORE : (i + 1) * N_PER_CORE], "ab": ab}
        for i in range(N_CORES)
    ]
    res = run_bass_kernel_spmd(nc, in_maps, core_ids=list(range(N_CORES)), trace=trace)
    outs = [
        np.asarray(res.results[i]["out"]).astype(np.float32) for i in range(N_CORES)
    ]
    return np.concatenate(outs, axis=0), res.exec_time_ns


def kernel(x: np.ndarray) -> np.ndarray:
    out, _ = run(x, trace=False)
    return out



# revision 36
# speedup vs baseline: 1.0218x; 1.0218x over previous
"""AntiAliasInterpolation2d Trainium kernel.

out[n,i,j,c] = sum_{dy,dx} g[dy]*g[dx] * x[n, 4i+dy-6, 4j+dx-6, c]   (zero pad)

i.e. a separable 13-tap Gaussian blur evaluated only on the stride-4 output
grid (the nearest-neighbor downsample of the reference picks blurred[4i,4j]).

Per core (batch shard of 4 images):
  vertical:   t1[i, w] = sum_h AB[h, i] * x[h, w]      (TensorE matmul,
              AB is the banded 512x128 matrix AB[h,i] = g[h-4i+6])
  narrow:     ACT copies t1 PSUM -> SBUF as bf16 into a tile with 8
              zero-guard columns each side (tolerance is 2e-2; bf16
              keeps rel err ~4e-3)
  horizontal: exploits kernel symmetry g[6-d]==g[6+d]:
                u_d[j]  = t1[4j-d] + t1[4j+d]   (6 DVE tensor_tensor adds,
                          bf16 2x mode; guards make every op full-range)
                out[j]  = g[6]*t1[4j] + sum_d g[6+d]*u_d[j]
                          (DVE: 4x-mode tensor_scalar + 6 stt MACs)
              accumulated in bf16 and DMA'd out bf16 (halves the output
              HBM write; the host widens back to f32)

Steady-state throughput is HBM-DMA-bound (~13.0 MB of HBM traffic per
core iteration at ~360 GB/s/core; measured 35.2-35.7us/iter vs a
35.3us bound at the 368 GB/s cost-model derate). For timing NEFFs
(repeats>1) the body holds up to 50 whole kernel executions per For_i
iteration: For_i places an all-engine barrier at each loop iteration,
and consecutive executions can only overlap (the tap tail hiding under
the next execution's input DMAs) within one loop body. The guard-zero
memsets run once in a prologue — the casts only write the tile
interior, so guards stay valid across iterations.

Things measured slower on HW than the baseline they modify:
  - pair-adds on Pool/GpSimd (n_pool>0): cost model says ~760ns/op but
    HW is far worse (42.9us vs 36.6us per iteration)
  - staggered_reset For_i instead of body unrolling (52.5us)
  - 13 plain stt taps straight from PSUM f32 (the previous scheme):
    DVE stream ~35.5us/iter paced right at the DMA wall (46.9us/iter
    with per-iteration barriers, 37.2us with ubody=16)

Built on bacc.Bacc: its generate_event_semaphores pass splits Tile's
multi-semaphore waits into EventSemaphore instructions (this walrus build
allows at most one semaphore wait per regular instruction).
"""

import numpy as np

try:
    import concourse.bass as bass
except ImportError:  # pragma: no cover
    import sys

    sys.path.insert(0, "/opt/trn_rl_repo")
    import concourse.bass as bass

import concourse.mybir as mybir
from concourse import bacc, tile
from concourse.bass_utils import run_bass_kernel_spmd

N_CORES = 8
N_PER_CORE = 4          # 32 images / 8 cores
H = W = 512
C = 3
OH = OW = 128
KSIZE = 13
KA = 6
SIGMA = 1.5
PAD = 8                 # zero-guard columns each side of the bf16 t1 copy
WPAD = W + 2 * PAD


def _gauss_norm() -> np.ndarray:
    r = np.arange(KSIZE, dtype=np.float32)
    g = np.exp(-((r - np.float32(KA)) ** 2) / np.float32(2.0 * SIGMA * SIGMA))
    return (g / g.sum()).astype(np.float32)


def _band_matrix() -> np.ndarray:
    """AB[h, i] = g[h - 4i + 6], zero outside the band."""
    g = _gauss_norm()
    ab = np.zeros((H, OH), dtype=np.float32)
    for i in range(OH):
        for dy in range(KSIZE):
            h = 4 * i + dy - KA
            if 0 <= h < H:
                ab[h, i] = g[dy]
    return ab


def _tap_ranges():
    """For each dx: (j0, j1, r, q) s.t. src w-index = 4*(j+q) + r for j in [j0,j1)."""
    taps = []
    for dx in range(KSIZE):
        off = dx - KA
        j0 = 0 if off >= 0 else (-off + 3) // 4  # ceil(-off/4)
        j1 = min(OW, (W - 1 - off) // 4 + 1)
        r = off % 4
        q = (off - r) // 4
        taps.append((dx, j0, j1, r, q))
    return taps


def build_nc(
    repeats: int = 1,
    n_chunks: int = 4,
    sbuf_taps: int = 0,
    dma_only: int = 0,
    tail_split: int = 0,
    unroll: int = 0,
    stag: int = 0,
    ubody: int = 0,
    obf: int = 1,
    split_taps: int = 1,
    n_pool: int = 0,
    rowpack: int = 0,
) -> bass.Bass:
    """repeats>1 re-runs the whole per-core program (for timing benchmarks).
    n_chunks: x DMAs per image (1, 2, or 4 h-blocks per DMA).
    sbuf_taps: copy t1 PSUM->SBUF on ACT first (measured slower on HW).
    dma_only: benchmark variant that skips all compute.
    tail_split: stream the LAST image in bank-aligned W-thirds so most of
      its tap work overlaps the final DMAs. Off by default: the DVE is
      still draining image 2's taps when image 3's early thirds land, so
      the split only adds op overhead (cost model: 52.5us vs 51.3us)."""
    nc = bacc.Bacc()
    f32 = mybir.dt.float32
    bf16 = mybir.dt.bfloat16
    # float32r: same 32-bit storage, but the PE streams it at 1 cycle/row
    # (plain float32 matmuls decompose into 2 half-rate passes = 4x slower)
    f32r = mybir.dt.float32r
    x = nc.declare_dram_parameter("x", [N_PER_CORE, H, W, C], f32r, isOutput=False)
    ab = nc.declare_dram_parameter("ab", [H, OH], f32r, isOutput=False)
    # obf: store the output as bf16 (halves the output HBM write traffic;
    # the host widens back to f32 — tolerance is 2e-2, bf16 err ~3e-3)
    out = nc.declare_dram_parameter(
        "out", [N_PER_CORE, OH, OW, C], bf16 if obf else f32, isOutput=True
    )

    g = _gauss_norm()
    taps = _tap_ranges()
    # full-coverage tap (dx=6) first so it can initialize the accumulator
    taps.sort(key=lambda t: t[0] != KA)

    with tile.TileContext(nc) as tc:
        with (
            tc.tile_pool(name="const", bufs=1) as cpool,
            tc.tile_pool(name="xp", bufs=1) as xpool,
            tc.tile_pool(name="op", bufs=1) as opool,
            tc.tile_pool(name="ps", bufs=2, space="PSUM") as pspool,
        ):
            # banded vertical matrix: sbuf [p=h%128, (k, i)] from dram
            # [(k p), i]; issued on the ACT HWDGE queue so it doesn't delay
            # the first x chunk at the head of the SP queue
            ab_s = cpool.tile([128, 4 * OH], f32r)
            nc.scalar.dma_start(
                out=ab_s[:].rearrange("p (k i) -> p k i", k=4),
                in_=ab.rearrange(
                    "(p k) i -> p k i" if rowpack else "(k p) i -> p k i",
                    p=128,
                ),
            )

            t1s_tiles = {}
            if split_taps and not dma_only:
                # one-time prologue: zero the guard columns of each image's
                # padded bf16 t1 tile. The per-iteration ACT casts only write
                # the interior, so the guards stay zero for the whole program.
                for n in range(N_PER_CORE):
                    t1s_tiles[n] = opool.tile(
                        [128, WPAD * C], bf16, tag=f"t1s{n}", name=f"t1s{n}"
                    )
                    nc.vector.memset(t1s_tiles[n][:, : PAD * C], 0.0)
                    nc.vector.memset(t1s_tiles[n][:, (PAD + W) * C :], 0.0)

            kb = 4 // n_chunks  # h-blocks per DMA

            def emit_image(n):
                # per-chunk DMAs: matmuls for a chunk start as soon as it
                # lands instead of waiting for the whole 3MB image;
                # dedicated tiles so the DMAs have no WAR deps
                xts = []
                for ck in range(n_chunks):
                    xtk = xpool.tile(
                        [128, kb * W * C], f32r, tag=f"xt{n}k{ck}", name=f"xt{n}k{ck}"
                    )
                    # rowpack: partition p holds kb CONSECUTIVE DRAM rows —
                    # one contiguous kb*6KB descriptor per partition instead
                    # of kb separate 6KB ones (ab is row-permuted to match)
                    src = (
                        "(ck p b) w c -> ck p b (w c)"
                        if rowpack
                        else "(ck b p) w c -> ck p b (w c)"
                    )
                    nc.sync.dma_start(
                        out=xtk[:].rearrange("p (b f) -> p b f", b=kb),
                        in_=x[n].rearrange(src, p=128, b=kb)[ck],
                    )
                    xts.append(xtk)

                if dma_only:
                    ot = opool.tile(
                        [128, OW * C], bf16 if obf else f32, tag=f"ot{n}", name=f"ot{n}"
                    )
                    nc.vector.tensor_copy(ot[:], xts[0][:, : OW * C].bitcast(f32))
                    nc.scalar.dma_start(
                        out=out[n].rearrange("i j c -> i (j c)"), in_=ot[:]
                    )
                    return

                # vertical blur via matmul, on the INTERLEAVED (w c) layout:
                # every column of x is blurred independently, so rhs can be
                # contiguous 512-element slices (PE streams at line rate;
                # strided rhs would throttle the XBUS). t1 free index is
                # m = w*3 + c.
                t1 = pspool.tile([128, C * W], f32, tag="t1", name=f"t1_{n}")
                for k in range(4):
                    lhsT = ab_s[:, k * OH : (k + 1) * OH]
                    xvk = xts[k // kb][:].rearrange("p (b f) -> p b f", b=kb)[
                        :, k % kb
                    ]
                    for s in range(C):
                        nc.tensor.matmul(
                            t1[:, s * W : (s + 1) * W],
                            lhsT,
                            xvk[:, s * W : (s + 1) * W],
                            start=(k == 0),
                            stop=(k == 3),
                        )

                if split_taps:
                    # Horizontal blur v2, exploiting the symmetric kernel
                    # (g[6-d] == g[6+d]) and a zero-padded bf16 copy of t1:
                    #   u_d[j] = t1[4j-d] + t1[4j+d]       (6 tensor_tensor
                    #       adds, split Pool/DVE; guard zeros make every op
                    #       full-range so there are no edge cases)
                    #   out[j] = g[6]*t1[4j] + sum_d g[6+d]*u_d[j]
                    #       (DVE: one 4x-mode tensor_scalar + 6 stt MACs,
                    #       accumulated in bf16 and DMA'd directly)
                    # ACT narrows t1 PSUM->SBUF bf16 (Pool has no PSUM port),
                    # one 512-wide PSUM-bank slice at a time.
                    t1s = t1s_tiles[n]
                    for s in range(C):
                        nc.scalar.copy(
                            t1s[:, PAD * C + s * W : PAD * C + (s + 1) * W],
                            t1[:, s * W : (s + 1) * W],
                        )

                    # [p, r, wq, c] with w_padded = 4*wq + r
                    t1r = t1s[:].rearrange("p (wq r c) -> p r wq c", r=4, c=C)

                    def src_at(off):
                        # view of t1[4j + off] for j in [0, OW)
                        wp = PAD + off
                        return t1r[:, wp % 4, wp // 4 : wp // 4 + OW]

                    acc = opool.tile(
                        [128, OW * C], bf16 if obf else f32,
                        tag=f"acc{n}", name=f"acc{n}",
                    )
                    av = acc[:].rearrange("p (j c) -> p j c", c=C)

                    us = {}
                    # n_pool = how many of the 6 pair-adds run on Pool
                    dve_pairs = tuple(range(1, 7 - n_pool))
                    for d in range(1, 7):
                        u = opool.tile([128, OW * C], bf16, tag=f"u{n}d{d}", name=f"u{n}d{d}")
                        eng = nc.vector if d in dve_pairs else nc.gpsimd
                        eng.tensor_tensor(
                            u[:].rearrange("p (j c) -> p j c", c=C),
                            src_at(-d),
                            src_at(d),
                            mybir.AluOpType.add,
                        )
                        us[d] = u

                    nc.vector.tensor_scalar(
                        av, src_at(0), float(g[KA]), None, mybir.AluOpType.mult
                    )
                    # DVE-computed pairs first so the chain rarely stalls on Pool
                    for d in list(dve_pairs) + [d for d in range(1, 7) if d not in dve_pairs]:
                        nc.vector.scalar_tensor_tensor(
                            acc[:],
                            us[d][:],
                            float(g[KA + d]),
                            acc[:],
                            mybir.AluOpType.mult,
                            mybir.AluOpType.add,
                        )
                    nc.scalar.dma_start(
                        out=out[n].rearrange("i j c -> i (j c)"), in_=acc[:]
                    )
                    return

                if sbuf_taps:
                    # PSUM -> SBUF via ACT so the DVE taps run all-SBUF
                    # (2x_2p mode in the cost model; measured slower on HW)
                    t1s = opool.tile(
                        [128, C * W], f32, tag=f"t1s{n}", name=f"t1s{n}"
                    )
                    nc.scalar.copy(t1s[:], t1[:])
                    tap_src = t1s
                else:
                    tap_src = t1

                # horizontal blur: 13 strided MACs on DVE
                # src index m = w*3 + c with w = 4u + r -> view [p, r, u, c]
                # (c innermost: each AP step covers a contiguous 12B triple)
                t1v = tap_src[:].rearrange("p (u r c) -> p r u c", r=4, c=C)
                ot = opool.tile([128, OW * C], f32, tag=f"ot{n}", name=f"ot{n}")
                ov = ot[:].rearrange("p (j c) -> p j c", c=C)

                first = True
                for dx, j0, j1, r, q in taps:
                    src = t1v[:, r, j0 + q : j1 + q]
                    dst = ov[:, j0:j1]
                    if first:
                        first = False
                        nc.vector.tensor_scalar(
                            dst, src, float(g[dx]), None, mybir.AluOpType.mult
                        )
                    else:
                        nc.vector.scalar_tensor_tensor(
                            dst,
                            src,
                            float(g[dx]),
                            dst,
                            mybir.AluOpType.mult,
                            mybir.AluOpType.add,
                        )

                # out DMA on the ACT HWDGE queue: its wait on the taps must
                # not block dispatch of later x DMAs on the SP queue
                if obf:
                    # narrow to bf16 on the (otherwise idle) ACT engine so the
                    # HBM write is half-width
                    otb = opool.tile([128, OW * C], bf16, tag=f"otb{n}", name=f"otb{n}")
                    nc.scalar.copy(otb[:], ot[:])
                    nc.scalar.dma_start(
                        out=out[n].rearrange("i j c -> i (j c)"), in_=otb[:]
                    )
                else:
                    nc.scalar.dma_start(
                        out=out[n].rearrange("i j c -> i (j c)"), in_=ot[:]
                    )

            def emit_image_tailsplit(n):
                # Last image of the stream: DMA it in 12 bank-aligned
                # W-thirds (third-major), and run the taps in two phases.
                # Phase A (j < 84) reads only PSUM banks 0-1 (m <= 1019)
                # so it overlaps the final third's DMAs + matmuls; only
                # phase B (j >= 84, ~1/3 of the tap work) trails the last
                # byte.
                JB = 84
                xts = {}
                for s in range(C):
                    for k in range(4):
                        t = xpool.tile(
                            [128, W], f32r, tag=f"xs{n}s{s}k{k}", name=f"xs{n}s{s}k{k}"
                        )
                        nc.sync.dma_start(
                            out=t[:],
                            in_=x[n].rearrange("(k p) w c -> k p (w c)", p=128)[k][
                                :, 512 * s : 512 * (s + 1)
                            ],
                        )
                        xts[(s, k)] = t

                t1 = pspool.tile([128, C * W], f32, tag="t1", name=f"t1_{n}")
                t1v = t1[:].rearrange("p (u r c) -> p r u c", r=4, c=C)
                ot = opool.tile([128, OW * C], f32, tag=f"ot{n}", name=f"ot{n}")
                ov = ot[:].rearrange("p (j c) -> p j c", c=C)

                def emit_taps(jlo, jhi):
                    first = True
                    for dx, j0, j1, r, q in taps:
                        jl, jh = max(j0, jlo), min(j1, jhi)
                        if jl >= jh:
                            continue
                        src = t1v[:, r, jl + q : jh + q]
                        dst = ov[:, jl:jh]
                        if first:
                            first = False
                            nc.vector.tensor_scalar(
                                dst, src, float(g[dx]), None, mybir.AluOpType.mult
                            )
                        else:
                            nc.vector.scalar_tensor_tensor(
                                dst,
                                src,
                                float(g[dx]),
                                dst,
                                mybir.AluOpType.mult,
                                mybir.AluOpType.add,
                            )

                for s in range(C):
                    for k in range(4):
                        nc.tensor.matmul(
                            t1[:, 512 * s : 512 * (s + 1)],
                            ab_s[:, k * OH : (k + 1) * OH],
                            xts[(s, k)][:],
                            start=(k == 0),
                            stop=(k == 3),
                        )
                    if s == 1:
                        emit_taps(0, JB)
                emit_taps(JB, OW)

                if obf:
                    otb = opool.tile([128, OW * C], bf16, tag=f"otb{n}", name=f"otb{n}")
                    nc.scalar.copy(otb[:], ot[:])
                    nc.scalar.dma_start(
                        out=out[n].rearrange("i j c -> i (j c)"), in_=otb[:]
                    )
                else:
                    nc.scalar.dma_start(
                        out=out[n].rearrange("i j c -> i (j c)"), in_=ot[:]
                    )

            def emit_all():
                for n in range(N_PER_CORE):
                    # tail_split = how many trailing images get the
                    # W-thirds streaming treatment
                    if dma_only or n < N_PER_CORE - tail_split:
                        emit_image(n)
                    else:
                        emit_image_tailsplit(n)

            if repeats == 1:
                emit_all()
            elif unroll:
                for _ in range(repeats):
                    emit_all()
            else:
                # unroll several whole kernel executions inside the For_i
                # body: For_i places an all-engine barrier at each loop
                # iteration, so consecutive executions only overlap (tail
                # under next execution's DMAs) within one body
                ub = ubody
                if ub == 0:
                    ub = next(
                        (u for u in (50, 32, 16, 8, 4, 2, 1) if repeats % u == 0), 1
                    )
                assert repeats % ub == 0, (repeats, ub)
                with tc.For_i(0, repeats // ub, 1, staggered_reset=bool(stag)):
                    for _ in range(ub):
                        emit_all()

    nc.finalize()
    return nc


_NC_CACHE = None


def _get_nc() -> bass.Bass:
    global _NC_CACHE
    if _NC_CACHE is None:
        _NC_CACHE = build_nc()
    return _NC_CACHE


def _spot_expected(x: np.ndarray, n: int, i: int, j: int) -> np.ndarray:
    """Host-side reference for one output pixel (all C channels)."""
    g = _gauss_norm()
    acc = np.zeros(C, np.float64)
    for dy in range(KSIZE):
        h = 4 * i + dy - KA
        if not (0 <= h < H):
            continue
        row = x[n, h]
        for dx in range(KSIZE):
            w = 4 * j + dx - KA
            if 0 <= w < W:
                acc += float(g[dy]) * float(g[dx]) * row[w]
    return acc.astype(np.float32)


def _spot_check(x: np.ndarray, out: np.ndarray) -> bool:
    """Cheap corruption detector: recompute a few pixels per image on host.

    Guards against a rare device-level transient (observed ~1 in 6 process
    launches returning a badly corrupted shard). bf16 rounding keeps honest
    outputs within ~1% of the spot values; corruption is O(1) off.
    """
    for n in range(out.shape[0]):
        for (i, j) in ((64, 64), (2, 126)):
            exp = _spot_expected(x, n, i, j)
            got = out[n, i, j]
            if np.any(np.abs(got - exp) > 0.03 * np.maximum(1.0, np.abs(exp))):
                return False
    return True


def run(x: np.ndarray, trace: bool = False):
    """Returns (out [32,128,128,3] f32, exec_time_ns or None)."""
    x = np.ascontiguousarray(np.asarray(x, dtype=np.float32))
    assert x.shape == (N_CORES * N_PER_CORE, H, W, C), x.shape
    ab = _band_matrix()
    nc = _get_nc()
    in_maps = [
        {"x": x[i * N_PER_CORE : (i + 1) * N_PER_CORE], "ab": ab}
        for i in range(N_CORES)
    ]
    for attempt in range(3):
        res = run_bass_kernel_spmd(
            nc, in_maps, core_ids=list(range(N_CORES)), trace=trace
        )
        out = np.concatenate(
            [
                np.asarray(res.results[i]["out"]).astype(np.float32)
                for i in range(N_CORES)
            ],
            axis=0,
        )
        if _spot_check(x, out):
            break
        print(f"kernel: spot-check failed (attempt {attempt}), re-running launch")
    return out, res.exec_time_ns


def kernel(x: np.ndarray) -> np.ndarray:
    out, _ = run(x, trace=False)
    return out

